# revision 9
# baseline (speedup 1.0000x reference)
"""Trainium2 Bass kernel for nn_HRNetW30classifier: logits = x @ W.T + b.

Shapes (full): x (8192, 2048) f32, W (1000, 2048) f32, b (1000,) f32
Output: (8192, 1000) f32.

Sharding: data-parallel over batch across 8 NeuronCores. Each core computes a
(1024, 2048) @ (2048, 1000) GEMM with W/b replicated.

Device kernel: host pre-transposes x and W so the contraction dim (K=2048)
lands on the SBUF partition axis (contiguous DMA rows). The TensorEngine runs
fp16 matmuls (1 col/cycle), accumulating fp32 in PSUM over 16 K-tiles.

Schedule (v3, tuned from traces; exec_time counts first-useful-op ->
last-teardown-op, with a fixed ~7us framework preamble excluded and a fixed
~8.8us semaphore-reset epilogue included):
- N=1000 splits into (512, 488) column chunks; each accumulation group is one
  PSUM bank. M=1024 splits into two mt-halves of 4.
- The dynamic-DMA path has ~2us queue spin-up + ~1us completion-semaphore
  latency, so the first operands are consumable only at ~10.4us while kernel
  code starts at ~6.8us. N_WARM scratch matmuls + the 2 bias-broadcast
  matmuls fill that window, keeping the PE busy so the HAM clock ramp
  (~5us of continuous activity to full rate) completes early in the real
  stream; any feed gap resets the ramp and costs ~2x matmul time until it
  re-ramps.
- b is sent as a single [1,1000] fp16 row (2KB, first in the DMA stream) and
  broadcast on the PE in the warmup window (ones[1,128].T @ b[1,N] -> PSUM),
  then copied to SBUF by the otherwise-idle Scalar engine. This keeps the
  0.5MB pre-broadcast bias tile out of the input stream, whose total bytes
  otherwise run neck-and-neck with the phase-2 x-half demand.
- Input DMA stream in phase-1 need-order: per kt only w[kt] + the phase-1
  x half (m 0:512); the phase-2 x half streams during phase-1 compute.
- Phase 1 (mt 0..3) is k-outer; its final k-step interleaves evictions per mt
  so PSUM banks are free before phase 2 (group-serial mt 4..7) needs them.
- Tail: evictions cost vec-add + ~600ns SP DMA-issue + transfer; the last
  group (mt7 n1) keeps a single DMA so the post-last-matmul chain is minimal.
"""

import numpy as np

P = 128
N_CORES = 8
B_FULL = 8192
M = B_FULL // N_CORES  # 1024 batch rows per core
N = 1000  # classes
K = 2048  # features
KT = K // P  # 16 k-tiles
MT = M // P  # 8 m-tiles
MH = MT // 2  # 4 m-tiles per phase
MHW = MH * P  # 512 batch cols in phase 1
N0_W = 512  # first n-chunk (one PSUM bank of fp32)
N1_W = N - N0_W  # 488

N_WARM_CONST = 8  # early warmup matmuls on the framework const tile (~6.6us)
N_WARM = 19  # scratch-tile warmup matmuls following them

MM_DTYPE = "fp16"  # "f32r" (TF32, ~2.4e-4) | "fp16" (~6e-4, fast) | "bf16" (~2e-3)

_NC_CACHE = {}


def _build_nc(mode=None):
    """Build + compile the per-core Bass program (SPMD: same NEFF on 8 cores)."""
    from contextlib import ExitStack

    import concourse.tile as tile
    from concourse import bacc, mybir
    from concourse._compat import get_trn_type

    mode = mode or MM_DTYPE
    f32 = mybir.dt.float32
    f32r = {
        "f32r": mybir.dt.float32r,
        "fp16": mybir.dt.float16,
        "bf16": mybir.dt.bfloat16,
    }[mode]

    nc = bacc.Bacc(get_trn_type() or "TRN2", target_bir_lowering=False, debug=False)

    xT = nc.dram_tensor("xT", [K, M], f32r, kind="ExternalInput")
    wT = nc.dram_tensor("wT", [K, N], f32r, kind="ExternalInput")
    bias = nc.dram_tensor("bias", [1, N], f32r, kind="ExternalInput")
    out = nc.dram_tensor("out", [M, N], f32, kind="ExternalOutput")

    xT_r = xT.ap().rearrange("(kt p) m -> kt p m", p=P)  # [KT, 128, M]
    wT_r = wT.ap().rearrange("(kt p) n -> kt p n", p=P)  # [KT, 128, N]
    out_r = out.ap().rearrange("(mt p) n -> mt p n", p=P)  # [MT, 128, N]

    with tile.TileContext(nc) as tc:
        with ExitStack() as ctx:
            xpool = ctx.enter_context(tc.tile_pool(name="xpool", bufs=1))
            wpool = ctx.enter_context(tc.tile_pool(name="wpool", bufs=1))
            bpool = ctx.enter_context(tc.tile_pool(name="bpool", bufs=1))
            opool = ctx.enter_context(tc.tile_pool(name="opool", bufs=8))
            pspool = ctx.enter_context(tc.tile_pool(name="ps", bufs=8, space="PSUM"))

            # Everything is resident in SBUF: x (64KB/part), W (62.5KB/part).
            x_sb = xpool.tile([P, KT, M], f32r, tag="x")
            w_sb = wpool.tile([P, KT, N], f32r, tag="w")
            wscr = bpool.tile([1, 256], f32r, tag="wscr")
            brow = bpool.tile([1, N], f32r, tag="brow")
            bias_t = bpool.tile([P, N], f32, tag="bias")

            # Input DMA stream in phase-1 need-order on the SP queue; the
            # tiny bias row rides the otherwise-idle Act queue in parallel
            # (a 600ns SP issue slot ahead of x/w delays first-data 1:1).
            # kt=0 is split fine and ordered for the kt0 consumption order
            # below (all n0 pairs, then all n1). Per kt, only w[kt] + the
            # phase-1 x half ride the early stream (~378KB/kt supply vs
            # >=1.7us/kt consumption), so phase 1 never starves after its
            # first pair; the phase-2 x half streams behind.
            nc.scalar.dma_start(brow[:], bias.ap())
            nc.sync.dma_start(x_sb[:, 0, 0:P], xT_r[0][:, 0:P])
            nc.sync.dma_start(w_sb[:, 0, 0:N0_W], wT_r[0][:, 0:N0_W])
            nc.sync.dma_start(x_sb[:, 0, P:MHW], xT_r[0][:, P:MHW])
            nc.sync.dma_start(w_sb[:, 0, N0_W:N], wT_r[0][:, N0_W:N])
            for kt in range(1, KT):
                nc.sync.dma_start(w_sb[:, kt, :], wT_r[kt])
                nc.sync.dma_start(x_sb[:, kt, 0:MHW], xT_r[kt][:, 0:MHW])
            for kt in range(KT):
                nc.sync.dma_start(x_sb[:, kt, MHW:M], xT_r[kt][:, MHW:M])

            # Keep the PE busy from kernel start until the first real
            # operands land, so the HAM clock-gate ramp runs continuously
            # into the real matmul stream (a feed gap resets it). The first
            # chunk reads the framework's const tile (initialized in the
            # Bass preamble, so it needs no memset of ours and starts
            # ~0.7us earlier); the rest use the scratch ones tile.
            ones_bf16 = nc.const_aps.aps[(mybir.dt.bfloat16, 1.0)]
            ps_w = pspool.tile([P, N0_W], f32, tag="ps", name="ps_warm")
            for _ in range(N_WARM_CONST):
                nc.tensor.matmul(
                    ps_w[0:1, 0:1],
                    lhsT=ones_bf16[0:1, 0:1],
                    rhs=ones_bf16[0:1, 0:1],
                    start=True,
                    stop=True,
                )
            nc.vector.memset(wscr[:], 1.0)
            for _ in range(N_WARM):
                nc.tensor.matmul(
                    ps_w[:, :128],
                    lhsT=wscr[:, 0:P],
                    rhs=wscr[:, 0:128],
                    start=True,
                    stop=True,
                )

            # Bias broadcast on the PE while still in the pre-data window:
            # ones[1,128].T @ b[1,N] fills PSUM with b replicated across
            # partitions; the idle Scalar engine copies it to SBUF. (fp16
            # carriage of b is exact for b=0 and ~1e-4 relative otherwise,
            # far inside the accuracy budget.)
            ps_ba = pspool.tile([P, N0_W], f32, tag="ps", name="ps_ba")
            ps_bb = pspool.tile([P, N0_W], f32, tag="ps", name="ps_bb")
            nc.tensor.matmul(
                ps_ba[:, :N0_W],
                lhsT=wscr[:, 0:P],
                rhs=brow[:, 0:N0_W],
                start=True,
                stop=True,
            )
            nc.tensor.matmul(
                ps_bb[:, :N1_W],
                lhsT=wscr[:, 0:P],
                rhs=brow[:, N0_W:N],
                start=True,
                stop=True,
            )
            nc.scalar.copy(bias_t[:, 0:N0_W], ps_ba[:, :N0_W])
            nc.scalar.copy(bias_t[:, N0_W:N], ps_bb[:, :N1_W])

            def mm_pair(psA, psB, mt, kt, start, stop):
                lhsT = x_sb[:, kt, mt * P : (mt + 1) * P]
                nc.tensor.matmul(
                    psA[:, :N0_W],
                    lhsT=lhsT,
                    rhs=w_sb[:, kt, 0:N0_W],
                    start=start,
                    stop=stop,
                )
                nc.tensor.matmul(
                    psB[:, :N1_W],
                    lhsT=lhsT,
                    rhs=w_sb[:, kt, N0_W:N],
                    start=start,
                    stop=stop,
                )

            def evict(ps_t, mt, n0, nw, add_eng=None, dma_eng=None):
                # Adds ride DVE and output DMA issues ride the Act queue by
                # default; the final eviction overrides both so its ~600ns
                # DMA issue and its bias-add run in parallel with the
                # second-to-last eviction's instead of queuing behind them.
                add_eng = add_eng or nc.vector
                dma_eng = dma_eng or nc.scalar
                ot = opool.tile([P, N0_W], f32, tag="ot", name=f"ot_{mt}_{n0}")
                add_eng.tensor_add(ot[:, :nw], ps_t[:, :nw], bias_t[:, n0 : n0 + nw])
                dma_eng.dma_start(out_r[mt, :, n0 : n0 + nw], ot[:, :nw])

            def ps_pair(mt):
                a = pspool.tile([P, N0_W], f32, tag="ps", name=f"psA_{mt}")
                b = pspool.tile([P, N0_W], f32, tag="ps", name=f"psB_{mt}")
                return a, b

            # ---- phase 1: mt 0..3, k-outer, paced by the DMA stream ----
            # kt=0 runs all n0 matmuls before the n1 ones so the four fine
            # kt0 input DMAs unblock consumption in arrival order.
            ps1 = [ps_pair(mt) for mt in range(MH)]
            for mt in range(MH):
                lhsT = x_sb[:, 0, mt * P : (mt + 1) * P]
                nc.tensor.matmul(
                    ps1[mt][0][:, :N0_W], lhsT=lhsT, rhs=w_sb[:, 0, 0:N0_W],
                    start=True, stop=False,
                )
            for mt in range(MH):
                lhsT = x_sb[:, 0, mt * P : (mt + 1) * P]
                nc.tensor.matmul(
                    ps1[mt][1][:, :N1_W], lhsT=lhsT, rhs=w_sb[:, 0, N0_W:N],
                    start=True, stop=False,
                )
            for kt in range(1, KT - 1):
                for mt in range(MH):
                    mm_pair(*ps1[mt], mt, kt, start=False, stop=False)
            # Final k-step interleaves evictions so PSUM banks free up while
            # the remaining mt pairs still run (phase 2 reuses them).
            for mt in range(MH):
                mm_pair(*ps1[mt], mt, KT - 1, start=False, stop=True)
                evict(ps1[mt][0], mt, 0, N0_W)
                evict(ps1[mt][1], mt, N0_W, N1_W)

            # ---- phase 2: mt 4..7, group-serial (x is SBUF-resident by
            # now); evictions stagger one group behind the matmuls ----
            for mt in range(MH, MT - 1):
                a, b = ps_pair(mt)
                for kt in range(KT):
                    mm_pair(a, b, mt, kt, start=(kt == 0), stop=(kt == KT - 1))
                evict(a, mt, 0, N0_W)
                evict(b, mt, N0_W, N1_W)

            # Last group (mt7): bias for the n1 half is pre-loaded into PSUM
            # by a 1-partition matmul, so the final eviction is a pure
            # PSUM->SBUF copy on the otherwise-idle Scalar engine, running in
            # parallel with DVE's n0 bias-add; the two output-DMA issues ride
            # different queues (Act / SP). This shortens the post-last-matmul
            # critical chain by ~1us for ~200ns of extra PE time.
            mt = MT - 1
            a, b = ps_pair(mt)
            nc.tensor.matmul(
                b[:, :N1_W], lhsT=wscr[:, 0:P], rhs=brow[:, N0_W:N],
                start=True, stop=False,
            )
            for kt in range(KT):
                lhsT = x_sb[:, kt, mt * P : (mt + 1) * P]
                nc.tensor.matmul(
                    a[:, :N0_W], lhsT=lhsT, rhs=w_sb[:, kt, 0:N0_W],
                    start=(kt == 0), stop=(kt == KT - 1),
                )
                nc.tensor.matmul(
                    b[:, :N1_W], lhsT=lhsT, rhs=w_sb[:, kt, N0_W:N],
                    start=False, stop=(kt == KT - 1),
                )
            evict(a, mt, 0, N0_W)
            ot_last = opool.tile([P, N0_W], f32, tag="ot", name="ot_last")
            nc.scalar.copy(ot_last[:, :N1_W], b[:, :N1_W])
            nc.sync.dma_start(out_r[mt, :, N0_W:N], ot_last[:, :N1_W])

    nc.compile()
    return nc


def _get_nc(mode=None):
    mode = mode or MM_DTYPE
    if mode not in _NC_CACHE:
        _NC_CACHE[mode] = _build_nc(mode)
    return _NC_CACHE[mode]


def _run(in_maps, trace=False, mode=None, **kwargs):
    from concourse.bass_utils import run_bass_kernel_spmd

    nc = _get_nc(mode)
    return run_bass_kernel_spmd(
        nc, in_maps, core_ids=list(range(N_CORES)), trace=trace, **kwargs
    )


def _round_tf32(a):
    """Round fp32 to the fp32r/TF32 grid (10 mantissa bits, RNE)."""
    u = np.ascontiguousarray(a, dtype=np.float32).view(np.uint32)
    r = u + 0x00000FFF + ((u >> 13) & 1)
    return (r & np.uint32(0xFFFFE000)).view(np.float32)


def _make_in_maps(x, W, b, mode=None):
    mode = mode or MM_DTYPE
    x = np.asarray(x, dtype=np.float32)
    W = np.asarray(W, dtype=np.float32)
    b = np.asarray(b, dtype=np.float32)
    if mode == "f32r":
        xT = _round_tf32(np.ascontiguousarray(x.T))  # (K, B_FULL)
        wT = _round_tf32(np.ascontiguousarray(W.T))  # (K, N)
        brow = _round_tf32(b[None, :])
    elif mode == "fp16":
        xT = np.ascontiguousarray(x.T).astype(np.float16)
        wT = np.ascontiguousarray(W.T).astype(np.float16)
        brow = b[None, :].astype(np.float16)
    else:
        import ml_dtypes

        xT = np.ascontiguousarray(x.T).astype(ml_dtypes.bfloat16)
        wT = np.ascontiguousarray(W.T).astype(ml_dtypes.bfloat16)
        brow = b[None, :].astype(ml_dtypes.bfloat16)
    return [
        {
            "xT": np.ascontiguousarray(xT[:, c * M : (c + 1) * M]),
            "wT": wT,
            "bias": np.ascontiguousarray(brow),
        }
        for c in range(N_CORES)
    ]


def kernel(x, W, b):
    res = _run(_make_in_maps(x, W, b))
    return np.concatenate([r["out"] for r in res.results], axis=0)


# revision 11
# speedup vs baseline: 1.0315x; 1.0315x over previous
"""Trainium2 Bass kernel for nn_HRNetW30classifier: logits = x @ W.T + b.

Shapes (full): x (8192, 2048) f32, W (1000, 2048) f32, b (1000,) f32
Output: (8192, 1000) f32.

Sharding: data-parallel over batch across 8 NeuronCores. Each core computes a
(1024, 2048) @ (2048, 1000) GEMM with W/b replicated.

Device kernel: host pre-transposes x and W so the contraction dim (K=2048)
lands on the SBUF partition axis (contiguous DMA rows). The TensorEngine runs
fp16 matmuls (1 col/cycle), accumulating fp32 in PSUM over 16 K-tiles.

Schedule (v3, tuned from traces; exec_time counts first-useful-op ->
last-teardown-op, with a fixed ~7us framework preamble excluded and a fixed
~8.8us semaphore-reset epilogue included):
- N=1000 splits into (512, 488) column chunks; each accumulation group is one
  PSUM bank. M=1024 splits into two mt-halves of 4.
- The dynamic-DMA path has ~2us queue spin-up + ~1us completion-semaphore
  latency, so the first operands are consumable only at ~10.4us while kernel
  code starts at ~6.8us. N_WARM scratch matmuls + the 2 bias-broadcast
  matmuls fill that window, keeping the PE busy so the HAM clock ramp
  (~5us of continuous activity to full rate) completes early in the real
  stream; any feed gap resets the ramp and costs ~2x matmul time until it
  re-ramps.
- b is sent as a single [1,1000] fp16 row (2KB, first in the DMA stream) and
  broadcast on the PE in the warmup window (ones[1,128].T @ b[1,N] -> PSUM),
  then copied to SBUF by the otherwise-idle Scalar engine. This keeps the
  0.5MB pre-broadcast bias tile out of the input stream, whose total bytes
  otherwise run neck-and-neck with the phase-2 x-half demand.
- Input DMA stream in phase-1 need-order: per kt only w[kt] + the phase-1
  x half (m 0:512); the phase-2 x half streams during phase-1 compute.
- Phase 1 (mt 0..3) is k-outer; its final k-step interleaves evictions per mt
  so PSUM banks are free before phase 2 (group-serial mt 4..7) needs them.
- Tail: evictions cost vec-add + ~600ns SP DMA-issue + transfer; the last
  group (mt7 n1) keeps a single DMA so the post-last-matmul chain is minimal.
"""

import numpy as np

P = 128
N_CORES = 8
B_FULL = 8192
M = B_FULL // N_CORES  # 1024 batch rows per core
N = 1000  # classes
K = 2048  # features
KT = K // P  # 16 k-tiles
MT = M // P  # 8 m-tiles
MH = MT // 2  # 4 m-tiles per phase
MHW = MH * P  # 512 batch cols in phase 1
N0_W = 512  # first n-chunk (one PSUM bank of fp32)
N1_W = N - N0_W  # 488

N_WARM_CONST = 20  # early 1x1 warmup matmuls (~26ns each) on the framework
# const tile: they start at PE kernel-entry (~7.2us) and bridge to when the
# scratch tile's memset semaphore clears (~7.7us) without a ramp-resetting gap
N_WARM = 18  # scratch-tile warmup matmuls (~107ns each) following them

MM_DTYPE = "fp16"  # "f32r" (TF32, ~2.4e-4) | "fp16" (~6e-4, fast) | "bf16" (~2e-3)

_NC_CACHE = {}


def _build_nc(mode=None):
    """Build + compile the per-core Bass program (SPMD: same NEFF on 8 cores)."""
    from contextlib import ExitStack

    import concourse.tile as tile
    from concourse import bacc, mybir
    from concourse._compat import get_trn_type

    mode = mode or MM_DTYPE
    f32 = mybir.dt.float32
    f32r = {
        "f32r": mybir.dt.float32r,
        "fp16": mybir.dt.float16,
        "bf16": mybir.dt.bfloat16,
    }[mode]

    nc = bacc.Bacc(get_trn_type() or "TRN2", target_bir_lowering=False, debug=False)

    xT = nc.dram_tensor("xT", [K, M], f32r, kind="ExternalInput")
    wT = nc.dram_tensor("wT", [K, N], f32r, kind="ExternalInput")
    bias = nc.dram_tensor("bias", [1, N], f32r, kind="ExternalInput")
    out = nc.dram_tensor("out", [M, N], f32, kind="ExternalOutput")

    xT_r = xT.ap().rearrange("(kt p) m -> kt p m", p=P)  # [KT, 128, M]
    wT_r = wT.ap().rearrange("(kt p) n -> kt p n", p=P)  # [KT, 128, N]
    out_r = out.ap().rearrange("(mt p) n -> mt p n", p=P)  # [MT, 128, N]

    with tile.TileContext(nc) as tc:
        with ExitStack() as ctx:
            xpool = ctx.enter_context(tc.tile_pool(name="xpool", bufs=1))
            wpool = ctx.enter_context(tc.tile_pool(name="wpool", bufs=1))
            bpool = ctx.enter_context(tc.tile_pool(name="bpool", bufs=1))
            opool = ctx.enter_context(tc.tile_pool(name="opool", bufs=8))
            pspool = ctx.enter_context(tc.tile_pool(name="ps", bufs=8, space="PSUM"))

            # Everything is resident in SBUF: x (64KB/part), W (62.5KB/part).
            x_sb = xpool.tile([P, KT, M], f32r, tag="x")
            w_sb = wpool.tile([P, KT, N], f32r, tag="w")
            wscr = bpool.tile([1, 256], f32r, tag="wscr")
            brow = bpool.tile([1, N], f32r, tag="brow")
            bias_t = bpool.tile([P, N], f32, tag="bias")

            # Input DMA stream in phase-1 need-order on the SP queue; the
            # tiny bias row rides the otherwise-idle Act queue in parallel
            # (a 600ns SP issue slot ahead of x/w delays first-data 1:1).
            # kt=0 is split fine and ordered for the kt0 consumption order
            # below (all n0 pairs, then all n1). Per kt, only w[kt] + the
            # phase-1 x half ride the early stream (~378KB/kt supply vs
            # >=1.7us/kt consumption), so phase 1 never starves after its
            # first pair; the phase-2 x half streams behind.
            nc.scalar.dma_start(brow[:], bias.ap())
            nc.sync.dma_start(x_sb[:, 0, 0:P], xT_r[0][:, 0:P])
            nc.sync.dma_start(w_sb[:, 0, 0:N0_W], wT_r[0][:, 0:N0_W])
            nc.sync.dma_start(x_sb[:, 0, P:MHW], xT_r[0][:, P:MHW])
            nc.sync.dma_start(w_sb[:, 0, N0_W:N], wT_r[0][:, N0_W:N])
            for kt in range(1, KT):
                nc.sync.dma_start(w_sb[:, kt, :], wT_r[kt])
                nc.sync.dma_start(x_sb[:, kt, 0:MHW], xT_r[kt][:, 0:MHW])
            for kt in range(KT):
                nc.sync.dma_start(x_sb[:, kt, MHW:M], xT_r[kt][:, MHW:M])

            # Keep the PE busy from kernel start until the first real
            # operands land, so the HAM clock-gate ramp runs continuously
            # into the real matmul stream (a feed gap resets it). The first
            # chunk reads the framework's const tile (initialized in the
            # Bass preamble, so it needs no memset of ours and starts
            # ~0.7us earlier); the rest use the scratch ones tile.
            ones_bf16 = nc.const_aps.aps[(mybir.dt.bfloat16, 1.0)]
            ps_w = pspool.tile([P, N0_W], f32, tag="ps", name="ps_warm")
            for _ in range(N_WARM_CONST):
                nc.tensor.matmul(
                    ps_w[0:1, 0:1],
                    lhsT=ones_bf16[0:1, 0:1],
                    rhs=ones_bf16[0:1, 0:1],
                    start=True,
                    stop=True,
                )
            nc.vector.memset(wscr[:], 1.0)
            for _ in range(N_WARM):
                nc.tensor.matmul(
                    ps_w[:, :128],
                    lhsT=wscr[:, 0:P],
                    rhs=wscr[:, 0:128],
                    start=True,
                    stop=True,
                )

            # Bias broadcast on the PE while still in the pre-data window:
            # ones[1,128].T @ b[1,N] fills PSUM with b replicated across
            # partitions; the idle Scalar engine copies it to SBUF. (fp16
            # carriage of b is exact for b=0 and ~1e-4 relative otherwise,
            # far inside the accuracy budget.)
            ps_ba = pspool.tile([P, N0_W], f32, tag="ps", name="ps_ba")
            ps_bb = pspool.tile([P, N0_W], f32, tag="ps", name="ps_bb")
            nc.tensor.matmul(
                ps_ba[:, :N0_W],
                lhsT=wscr[:, 0:P],
                rhs=brow[:, 0:N0_W],
                start=True,
                stop=True,
            )
            nc.tensor.matmul(
                ps_bb[:, :N1_W],
                lhsT=wscr[:, 0:P],
                rhs=brow[:, N0_W:N],
                start=True,
                stop=True,
            )
            nc.scalar.copy(bias_t[:, 0:N0_W], ps_ba[:, :N0_W])
            nc.scalar.copy(bias_t[:, N0_W:N], ps_bb[:, :N1_W])

            def mm_pair(psA, psB, mt, kt, start, stop):
                lhsT = x_sb[:, kt, mt * P : (mt + 1) * P]
                nc.tensor.matmul(
                    psA[:, :N0_W],
                    lhsT=lhsT,
                    rhs=w_sb[:, kt, 0:N0_W],
                    start=start,
                    stop=stop,
                )
                nc.tensor.matmul(
                    psB[:, :N1_W],
                    lhsT=lhsT,
                    rhs=w_sb[:, kt, N0_W:N],
                    start=start,
                    stop=stop,
                )

            def evict(ps_t, mt, n0, nw, add_eng=None, dma_eng=None):
                # Adds ride DVE and output DMA issues ride the Act queue by
                # default; the final eviction overrides both so its ~600ns
                # DMA issue and its bias-add run in parallel with the
                # second-to-last eviction's instead of queuing behind them.
                add_eng = add_eng or nc.vector
                dma_eng = dma_eng or nc.scalar
                ot = opool.tile([P, N0_W], f32, tag="ot", name=f"ot_{mt}_{n0}")
                add_eng.tensor_add(ot[:, :nw], ps_t[:, :nw], bias_t[:, n0 : n0 + nw])
                dma_eng.dma_start(out_r[mt, :, n0 : n0 + nw], ot[:, :nw])

            def ps_pair(mt):
                a = pspool.tile([P, N0_W], f32, tag="ps", name=f"psA_{mt}")
                b = pspool.tile([P, N0_W], f32, tag="ps", name=f"psB_{mt}")
                return a, b

            # ---- phase 1: mt 0..3, k-outer, paced by the DMA stream ----
            # kt=0 runs all n0 matmuls before the n1 ones so the four fine
            # kt0 input DMAs unblock consumption in arrival order.
            ps1 = [ps_pair(mt) for mt in range(MH)]
            for mt in range(MH):
                lhsT = x_sb[:, 0, mt * P : (mt + 1) * P]
                nc.tensor.matmul(
                    ps1[mt][0][:, :N0_W], lhsT=lhsT, rhs=w_sb[:, 0, 0:N0_W],
                    start=True, stop=False,
                )
            for mt in range(MH):
                lhsT = x_sb[:, 0, mt * P : (mt + 1) * P]
                nc.tensor.matmul(
                    ps1[mt][1][:, :N1_W], lhsT=lhsT, rhs=w_sb[:, 0, N0_W:N],
                    start=True, stop=False,
                )
            for kt in range(1, KT - 1):
                for mt in range(MH):
                    mm_pair(*ps1[mt], mt, kt, start=False, stop=False)
            # Final k-step interleaves evictions so PSUM banks free up while
            # the remaining mt pairs still run (phase 2 reuses them).
            for mt in range(MH):
                mm_pair(*ps1[mt], mt, KT - 1, start=False, stop=True)
                evict(ps1[mt][0], mt, 0, N0_W)
                evict(ps1[mt][1], mt, N0_W, N1_W)

            # ---- phase 2: mt 4..7, group-serial (x is SBUF-resident by
            # now); evictions stagger one group behind the matmuls ----
            for mt in range(MH, MT - 1):
                a, b = ps_pair(mt)
                for kt in range(KT):
                    mm_pair(a, b, mt, kt, start=(kt == 0), stop=(kt == KT - 1))
                evict(a, mt, 0, N0_W)
                evict(b, mt, N0_W, N1_W)

            # Last group (mt7): bias for the n1 half is pre-loaded into PSUM
            # by a 1-partition matmul, so the final eviction is a pure
            # PSUM->SBUF copy on the otherwise-idle Scalar engine, running in
            # parallel with DVE's n0 bias-add; the two output-DMA issues ride
            # different queues (Act / SP). This shortens the post-last-matmul
            # critical chain by ~1us for ~200ns of extra PE time.
            # The two groups run as sequential k-loops (LDWEIGHTS is emitted
            # per-matmul anyway, so re-streaming the stationary tiles is
            # free): n0 finishes 16 matmuls early and its 256KB eviction
            # fully overlaps n1's k-loop, leaving only n1's 244KB to move
            # after the final matmul.
            mt = MT - 1
            a, b = ps_pair(mt)
            for kt in range(KT):
                nc.tensor.matmul(
                    a[:, :N0_W],
                    lhsT=x_sb[:, kt, mt * P : (mt + 1) * P],
                    rhs=w_sb[:, kt, 0:N0_W],
                    start=(kt == 0), stop=(kt == KT - 1),
                )
            evict(a, mt, 0, N0_W)
            nc.tensor.matmul(
                b[:, :N1_W], lhsT=wscr[:, 0:P], rhs=brow[:, N0_W:N],
                start=True, stop=False,
            )
            for kt in range(KT):
                nc.tensor.matmul(
                    b[:, :N1_W],
                    lhsT=x_sb[:, kt, mt * P : (mt + 1) * P],
                    rhs=w_sb[:, kt, N0_W:N],
                    start=False, stop=(kt == KT - 1),
                )
            ot_last = opool.tile([P, N0_W], f32, tag="ot", name="ot_last")
            nc.scalar.copy(ot_last[:, :N1_W], b[:, :N1_W])
            nc.sync.dma_start(out_r[mt, :, N0_W:N], ot_last[:, :N1_W])

    nc.compile()
    return nc


def _get_nc(mode=None):
    mode = mode or MM_DTYPE
    if mode not in _NC_CACHE:
        _NC_CACHE[mode] = _build_nc(mode)
    return _NC_CACHE[mode]


def _run(in_maps, trace=False, mode=None, **kwargs):
    from concourse.bass_utils import run_bass_kernel_spmd

    nc = _get_nc(mode)
    return run_bass_kernel_spmd(
        nc, in_maps, core_ids=list(range(N_CORES)), trace=trace, **kwargs
    )


def _round_tf32(a):
    """Round fp32 to the fp32r/TF32 grid (10 mantissa bits, RNE)."""
    u = np.ascontiguousarray(a, dtype=np.float32).view(np.uint32)
    r = u + 0x00000FFF + ((u >> 13) & 1)
    return (r & np.uint32(0xFFFFE000)).view(np.float32)


def _make_in_maps(x, W, b, mode=None):
    mode = mode or MM_DTYPE
    x = np.asarray(x, dtype=np.float32)
    W = np.asarray(W, dtype=np.float32)
    b = np.asarray(b, dtype=np.float32)
    if mode == "f32r":
        xT = _round_tf32(np.ascontiguousarray(x.T))  # (K, B_FULL)
        wT = _round_tf32(np.ascontiguousarray(W.T))  # (K, N)
        brow = _round_tf32(b[None, :])
    elif mode == "fp16":
        xT = np.ascontiguousarray(x.T).astype(np.float16)
        wT = np.ascontiguousarray(W.T).astype(np.float16)
        brow = b[None, :].astype(np.float16)
    else:
        import ml_dtypes

        xT = np.ascontiguousarray(x.T).astype(ml_dtypes.bfloat16)
        wT = np.ascontiguousarray(W.T).astype(ml_dtypes.bfloat16)
        brow = b[None, :].astype(ml_dtypes.bfloat16)
    return [
        {
            "xT": np.ascontiguousarray(xT[:, c * M : (c + 1) * M]),
            "wT": wT,
            "bias": np.ascontiguousarray(brow),
        }
        for c in range(N_CORES)
    ]


def kernel(x, W, b):
    res = _run(_make_in_maps(x, W, b))
    return np.concatenate([r["out"] for r in res.results], axis=0)
